# revision 7
# baseline (speedup 1.0000x reference)
"""Trainium2 Bass kernel for Mistral-style attention with an INVERTED band mask.

Reference semantics (S=2048, E=4096, H=32, KV=8, D=128, WINDOW=1024):
  q/k/v projections -> RoPE(q,k) -> GQA attention where positions with
  |i-j| < 1024 are masked OUT (attend only to far positions) -> softmax ->
  out projection.

Sharding (8 cores, tensor-parallel by GQA group):
  core c owns KV head c and Q heads 4c..4c+3. Column-parallel QKV,
  row-parallel O projection; the 8 fp16 partial outputs are summed on host.

Schedule (v2: baseline 572us -> 373us -> this):
  - Startup: the first matmul needs only e-tile 0 of hid + weights. Groups
    0-1 of the first chunk's hid/wq/wk/wv are DMA'd as per-e-tile tiles,
    interleaved across four DMA queues (scalar=hid, sync=wq, vector=wk/wv,
    gpsimd=bulk), so the first MM starts ~9us (vs ~20us with 1MB tiles)
    and the stream stays fed at 0.32MB per 1.28us e-tile.
  - Per-chunk qT/kT/v/attn tiles: tile-granular dependency tracking made
    attention chunk 3 wait on the LAST chunk's rope writes into the shared
    qT/kT tiles; separate per-chunk tiles remove the false dependency.
  - Single-pass QKV projection: Q PSUM split into two [128,2,512] tiles
    (bufs=3) + psk/psv (bufs=1) = 8 banks, so chunk n+1's matmuls start
    while chunk n drains (on ScalarE, idle during phase 1).
  - Phase-1 chunk order [3,0,1,2] and attention order [3,2,1,0]: no
    attention chunk ever waits on the last-projected chunk's RoPE tail.
    The final chunk drains h2/h3 first, split across ScalarE+DVE, so the
    PSUM banks the first score matmuls reuse free in ~1.6us.
  - Softmax denominator per score block via an all-ones [128,128]
    stationary matmul accumulating into a [128,512] PSUM tile - the
    partition broadcast is free, so normalize is just approx-reciprocal
    + multiply on DVE.
  - Ragged attention-V / denominator matmuls: start=True only on the first
    entry, stop on the last.
  - O projection of the previous chunk interleaved after each attention
    chunk; output rows DMA out in quarters to shorten the kernel tail.
"""

import math
from contextlib import ExitStack

import numpy as np
import ml_dtypes

import concourse.bass as bass
import concourse.mybir as mybir
import concourse.tile as tile
from concourse import bacc
from concourse.bass_utils import run_bass_kernel_spmd

P = 128
S = 2048
E = 4096
D = 128
HPC = 4          # q heads per core
NE = E // P      # 32 e-tiles
NSCH = 4         # s-chunks of 512
SCH = S // NSCH  # 512
NST = S // P     # 16 s-tiles
NEO = 8          # output e-chunks of 512
GE = 8           # e-tiles per hid DMA group
NG = NE // GE    # 4 groups
NF = 2 * GE      # fine-grained e-tiles at startup (groups 0-1)
SCALE = 1.0 / math.sqrt(D)
F16 = mybir.dt.float16
F32 = mybir.dt.float32
BF16 = mybir.dt.bfloat16


def _allowed_tiles(c):
    """For s-chunk c (query blocks bi=4c..4c+3), list (bj, lo, hi, mask, mpos):
    key tile bj is needed for query sub-tiles [lo, hi) (chunk-relative);
    mask in {None,'low','up'} applied at chunk-relative position mpos."""
    out = []
    bis = range(4 * c, 4 * c + 4)
    for bj in range(NST):
        ok = [bi for bi in bis if abs(bi - bj) >= 8]
        if not ok:
            continue
        lo = min(ok) - 4 * c
        hi = max(ok) + 1 - 4 * c
        assert ok == list(range(lo + 4 * c, hi + 4 * c)), (c, bj, ok)
        mask, mpos = None, 0
        if bj - 8 in ok:
            mask, mpos = "low", bj - 8 - 4 * c
        elif bj + 8 in ok:
            mask, mpos = "up", bj + 8 - 4 * c
        out.append((bj, lo, hi, mask, mpos))
    return out


def build_nc(debug=False):
    nc = bacc.Bacc("TRN2", target_bir_lowering=False, debug=False)
    # host-relaid tensors: partition-major, contiguous per partition
    hidw = nc.dram_tensor("hidw", (P, NSCH * NG, GE * SCH), F16,
                          kind="ExternalInput")
    wqw = nc.dram_tensor("wqw", (P, NG, GE * HPC * D), F16,
                         kind="ExternalInput")
    wkvw = nc.dram_tensor("wkvw", (P, NG, GE * 2 * D), F16,
                          kind="ExternalInput")
    wow = nc.dram_tensor("wow", (P, HPC * E), F16, kind="ExternalInput")
    cosT = nc.dram_tensor("cosT", (D, S), F16, kind="ExternalInput")
    sinT = nc.dram_tensor("sinT", (D, S), F16, kind="ExternalInput")
    mlow = nc.dram_tensor("mlow", (P, P), BF16, kind="ExternalInput")
    mup = nc.dram_tensor("mup", (P, P), BF16, kind="ExternalInput")
    outd = nc.dram_tensor("out", (S, E), F16, kind="ExternalOutput")

    with tile.TileContext(nc) as tc, ExitStack() as ctx:
        const = ctx.enter_context(tc.tile_pool(name="const", bufs=1))

        # groups 0-1 weights live in per-e-tile tiles (fine-grained startup
        # DMA); groups 2-3 in bulk per-group tiles.
        wq_f = [const.tile([P, HPC * D], F16, name=f"wqf{e}")
                for e in range(NF)]
        wkv_f = [const.tile([P, 2 * D], F16, name=f"wkvf{e}")
                 for e in range(NF)]
        wq_g = {g: const.tile([P, GE, HPC * D], F16, name=f"wq{g}")
                for g in (2, 3)}
        wkv_g = {g: const.tile([P, GE, 2 * D], F16, name=f"wkv{g}")
                 for g in (2, 3)}
        wo_sb = const.tile([P, HPC, E], F16)
        cos_sb = const.tile([P, S], F16)
        sin_sb = const.tile([P, S], F16)
        ml_sb = const.tile([P, P], BF16)
        mu_sb = const.tile([P, P], BF16)
        ones_sb = const.tile([P, P], BF16)

        # per-chunk roped Q^T / K^T / V / attention-output tiles (separate
        # tiles so tile-granular dep tracking can't create false waits)
        qT_c = [const.tile([P, HPC, SCH], F16, name=f"qT{c}")
                for c in range(NSCH)]
        kT_c = [const.tile([P, SCH], F16, name=f"kT{c}") for c in range(NSCH)]
        v_c = [const.tile([P, 4, D], F16, name=f"v{c}") for c in range(NSCH)]
        attn_c = [const.tile([P, HPC, SCH], F16, name=f"at{c}")
                  for c in range(NSCH)]

        hidp = ctx.enter_context(tc.tile_pool(name="hid", bufs=3))
        rp = ctx.enter_context(tc.tile_pool(name="rope", bufs=2))

        # phase-1 projects chunks in this order; attention runs [3,2,1,0],
        # so no attention chunk waits on the last-projected chunk's ropes.
        ph1_order = [3, 0, 1, 2]
        c_first = ph1_order[0]

        def wdma(eng, dst, src, g):
            eng.dma_start(dst[:], src[:, g, :].rearrange(
                "p (ge d) -> p ge d", ge=GE))

        # ---- startup DMA: fine-grained first parcel -----------------------
        # e-tile e of the first chunk needs hid[e] (128KB) + wq[e] (128KB)
        # + wk/wv[e] (32KB each); with per-e-tile tiles on four queues the
        # first MM starts as soon as ~0.32MB lands (~1.5us of transfer)
        # instead of waiting for 2.6MB of monolithic tiles.
        hid_f = []
        for e in range(NF):
            g, ee = divmod(e, GE)
            htf = hidp.tile([P, SCH], F16, tag="hidf", bufs=8)
            nc.scalar.dma_start(
                htf[:], hidw[:, c_first * NG + g, ee * SCH:(ee + 1) * SCH])
            hid_f.append(htf)
            nc.sync.dma_start(
                wq_f[e][:], wqw[:, g, ee * HPC * D:(ee + 1) * HPC * D])
            nc.gpsimd.dma_start(
                wkv_f[e][:], wkvw[:, g, ee * 2 * D:(ee + 1) * 2 * D])
        nc.sync.dma_start(ml_sb[:], mlow[:])
        nc.sync.dma_start(mu_sb[:], mup[:])
        # bulk weights for groups 2-3 behind the fine pieces
        wdma(nc.sync, wq_g[2], wqw, 2)
        wdma(nc.sync, wq_g[3], wqw, 3)
        wdma(nc.gpsimd, wkv_g[2], wkvw, 2)
        wdma(nc.gpsimd, wkv_g[3], wkvw, 3)
        nc.sync.dma_start(cos_sb[:], cosT[:])
        nc.sync.dma_start(sin_sb[:], sinT[:])
        nc.gpsimd.memset(ones_sb[:], 1.0)

        def rope_k_rest(kraw, c):
            csl = slice(c * SCH, (c + 1) * SCH)
            krot = rp.tile([P, SCH], F16, tag="krot", bufs=2)
            nc.sync.dma_start(krot[0:64, :], kraw[64:128, :])
            nc.sync.dma_start(krot[64:128, :], kraw[0:64, :])
            nc.vector.tensor_tensor(
                kraw[:], kraw[:], cos_sb[:, csl], mybir.AluOpType.mult)
            nc.vector.tensor_tensor(
                krot[:], krot[:], sin_sb[:, csl], mybir.AluOpType.mult)
            nc.vector.tensor_tensor(
                kT_c[c][:], kraw[:], krot[:], mybir.AluOpType.add)

        def rope_q_head(psq1, h, c, drain_eng=None):
            """psq1 [P, SCH] = pre-rope head h of chunk c."""
            csl = slice(c * SCH, (c + 1) * SCH)
            qraw = rp.tile([P, SCH], F16, tag="qraw", bufs=4)
            if drain_eng == "vector":
                nc.vector.tensor_copy(qraw[:], psq1)
            else:
                nc.scalar.copy(qraw[:], psq1)
            qrot = rp.tile([P, SCH], F16, tag="qrot", bufs=4)
            nc.sync.dma_start(qrot[0:64, :], qraw[64:128, :])
            nc.sync.dma_start(qrot[64:128, :], qraw[0:64, :])
            nc.vector.tensor_tensor(
                qraw[:], qraw[:], cos_sb[:, csl], mybir.AluOpType.mult)
            nc.vector.tensor_tensor(
                qrot[:], qrot[:], sin_sb[:, csl], mybir.AluOpType.mult)
            nc.vector.tensor_tensor(
                qT_c[c][:, h, :], qraw[:], qrot[:], mybir.AluOpType.add)

        # ---- Phase 1: QKV projections (+RoPE) ----
        with tc.tile_pool(name="p1psum", bufs=1, space="PSUM") as p1:
            for ci, c in enumerate(ph1_order):
                psqA = p1.tile([P, 2, SCH], F32, tag="psq2", bufs=3)
                psqB = p1.tile([P, 2, SCH], F32, tag="psq2", bufs=3)
                psk = p1.tile([P, SCH], F32, tag="psk", bufs=1)
                psv = p1.tile([P, SCH], F32, tag="psv", bufs=1)
                ht = None
                for e in range(NE):
                    g, ee = divmod(e, GE)
                    fine = (ci == 0 and g < 2)
                    if fine:
                        rhs = hid_f[e][:]
                    else:
                        if ee == 0:
                            ht = hidp.tile([P, GE * SCH], F16, tag="hid")
                            nc.gpsimd.dma_start(ht[:], hidw[:, c * NG + g, :])
                        rhs = ht[:, ee * SCH:(ee + 1) * SCH]
                    if g < 2:
                        wq_ap = wq_f[e]
                        wk_ap = wkv_f[e][:, 0:D]
                        wv_ap = wkv_f[e][:, D:2 * D]
                    else:
                        wq_ap = wq_g[g][:, ee]
                        wk_ap = wkv_g[g][:, ee, 0:D]
                        wv_ap = wkv_g[g][:, ee, D:2 * D]
                    st = (e == 0)
                    sp = (e == NE - 1)
                    for h in range(HPC):
                        dst = psqA[:, h, :] if h < 2 else psqB[:, h - 2, :]
                        nc.tensor.matmul(
                            dst, wq_ap[:, h * D:(h + 1) * D], rhs,
                            start=st, stop=sp)
                    nc.tensor.matmul(psk[:], wk_ap, rhs, start=st, stop=sp)
                    nc.tensor.matmul(psv[:], wv_ap, rhs, start=st, stop=sp)
                # Drain order on ScalarE: mid-pipeline chunks free the
                # single-buffered psk/psv first; the final chunk frees the
                # Q banks first (attention's PSUM pool reuses them and no
                # successor chunk needs psk/psv).
                kraw = rp.tile([P, SCH], F16, tag="kraw", bufs=2)
                vstage = rp.tile([P, SCH], F16, tag="vstage", bufs=2)
                if ci < NSCH - 1:
                    nc.scalar.copy(kraw[:], psk[:])
                    nc.scalar.copy(vstage[:], psv[:])
                    rope_k_rest(kraw, c)
                    for j, ps in enumerate((psqA, psqA, psqB, psqB)):
                        rope_q_head(ps[:, j % 2, :], j, c)
                else:
                    # final chunk: drain h2/h3 first (their banks become the
                    # attention score tiles), split across ACT and DVE so
                    # all four banks free in ~1.6us instead of 4us
                    for j in (2, 3, 0, 1):
                        ps = psqB if j >= 2 else psqA
                        rope_q_head(ps[:, j % 2, :], j, c,
                                    drain_eng="vector" if j % 2 else None)
                    nc.scalar.copy(kraw[:], psk[:])
                    nc.scalar.copy(vstage[:], psv[:])
                    rope_k_rest(kraw, c)
                nc.sync.dma_start_transpose(v_c[c][:], vstage[:])
            # wo behind all hid tiles on the GpSimd queue: lands right
            # after phase 1 ends, just before the first O-projection.
            nc.gpsimd.dma_start(
                wo_sb[:], wow.rearrange("p (ho e) -> p ho e", ho=HPC))

        # ---- Phase 2+3: attention, O-projection of chunk c-1 interleaved ----
        ep = ctx.enter_context(tc.tile_pool(name="expp", bufs=3))
        np_pool = ctx.enter_context(tc.tile_pool(name="normp", bufs=2))
        osp = ctx.enter_context(tc.tile_pool(name="ostage", bufs=2))

        with tc.tile_pool(name="apsum", bufs=2, space="PSUM") as ap:
            def attn_chunk(c):
                entries = _allowed_tiles(c)
                nblk = len(entries)
                for h in range(HPC):
                    psa = ap.tile([P, SCH], F32, tag="psa")
                    psd = ap.tile([P, SCH], F32, tag="psd")
                    etsum = ep.tile([P, SCH], BF16, tag="etsum", bufs=2)
                    nc.gpsimd.memset(etsum[:], 0.0)
                    for idx, (bj, lo, hi, mask, mpos) in enumerate(entries):
                        n = (hi - lo) * P
                        pss = ap.tile([P, SCH], F32, tag="pss")
                        nc.tensor.matmul(
                            pss[:, :n],
                            kT_c[bj // 4][:, (bj % 4) * P:(bj % 4 + 1) * P],
                            qT_c[c][:, h, lo * P:hi * P],
                            start=True, stop=True)
                        et = ep.tile([P, SCH], BF16, tag="et")
                        nc.scalar.activation(
                            et[:, :n], pss[:, :n],
                            mybir.ActivationFunctionType.Exp, scale=SCALE)
                        if mask is not None:
                            msb = ml_sb if mask == "low" else mu_sb
                            nc.vector.tensor_tensor(
                                et[:, (mpos - lo) * P:(mpos - lo + 1) * P],
                                et[:, (mpos - lo) * P:(mpos - lo + 1) * P],
                                msb[:], mybir.AluOpType.mult)
                        nc.tensor.matmul(
                            psa[:, lo * P:hi * P],
                            v_c[bj // 4][:, bj % 4, :], et[:, :n],
                            start=(idx == 0), stop=(idx == nblk - 1),
                            skip_group_check=True)
                        # off the AV critical path: only the one denominator
                        # matmul at the end of the head waits on this chain
                        nc.vector.tensor_tensor(
                            etsum[:, lo * P:hi * P], etsum[:, lo * P:hi * P],
                            et[:, :n], mybir.AluOpType.add)
                    nc.tensor.matmul(
                        psd[:], ones_sb[:], etsum[:], start=True, stop=True)
                    rcp = np_pool.tile([P, SCH], F32, tag="rcp")
                    nc.vector.reciprocal_approx_fast(rcp[:], psd[:])
                    nc.vector.tensor_tensor(
                        attn_c[c][:, h, :], psa[:], rcp[:],
                        mybir.AluOpType.mult)

            def oproj_chunk(c):
                for j in range(4):
                    st = 4 * c + j
                    orow = osp.tile([P, E], F16, tag="orow")
                    for eo in range(NEO):
                        pso = ap.tile([P, SCH], F32, tag="pso")
                        for h in range(HPC):
                            nc.tensor.matmul(
                                pso[:],
                                attn_c[c][:, h, j * P:(j + 1) * P],
                                wo_sb[:, h, eo * SCH:(eo + 1) * SCH],
                                start=(h == 0), stop=(h == HPC - 1))
                        if eo % 2 == 0:
                            nc.vector.tensor_copy(
                                orow[:, eo * SCH:(eo + 1) * SCH], pso[:])
                        else:
                            nc.scalar.copy(
                                orow[:, eo * SCH:(eo + 1) * SCH], pso[:])
                            q = eo // 2
                            nc.sync.dma_start(
                                outd[st * P:(st + 1) * P,
                                     q * 2 * SCH:(q + 1) * 2 * SCH],
                                orow[:, q * 2 * SCH:(q + 1) * 2 * SCH])

            # chunk order 3,2,1,0: chunk 3's keys (bj 0..7) are roped first
            # in phase 1, so attention starts without waiting for chunk 3's
            # K rope; O-projection trails attention by one chunk.
            order = [3, 2, 1, 0]
            attn_chunk(order[0])
            for i in range(1, NSCH):
                attn_chunk(order[i])
                oproj_chunk(order[i - 1])
            oproj_chunk(order[-1])
    nc.compile()
    return nc


_NC_CACHE = {}


def get_nc():
    if "nc" not in _NC_CACHE:
        _NC_CACHE["nc"] = build_nc()
    return _NC_CACHE["nc"]


def make_in_maps(hidden_states, Wq, Wk, Wv, Wo):
    hid = np.asarray(hidden_states).reshape(S, E)
    hidT16 = np.ascontiguousarray(hid.T).astype(np.float16)   # [E, S]
    # [p, c, g, ee, s'] contiguous per partition
    hidw = (hidT16.reshape(NG, GE, P, NSCH, SCH)
            .transpose(2, 3, 0, 1, 4)
            .reshape(P, NSCH * NG, GE * SCH))
    hidw = np.ascontiguousarray(hidw)

    inv = 1.0 / (10000.0 ** (np.arange(0, D, 2, dtype=np.float64) / D))
    t = np.arange(S, dtype=np.float64)
    fr = np.outer(t, inv)                      # [S, 64]
    emb = np.concatenate([fr, fr], axis=1)     # [S, 128]
    cosT = np.ascontiguousarray(np.cos(emb).T).astype(np.float16)
    sinT = np.sin(emb).T.copy()
    sinT[:64] *= -1.0                          # rotate_half sign fold
    sinT = np.ascontiguousarray(sinT).astype(np.float16)

    jj = np.arange(P)[:, None]
    ii = np.arange(P)[None, :]
    mlow = (jj >= ii).astype(ml_dtypes.bfloat16)   # block bj-bi=8: j-i>=1024
    mup = (ii >= jj).astype(ml_dtypes.bfloat16)    # block bi-bj=8: i-j>=1024

    def wlayout(w, inner):
        # w [E_in, cols] -> [p, g, ee, cols] contiguous per partition
        arr = np.ascontiguousarray(w.T).astype(np.float16)   # [E_in, cols]
        return np.ascontiguousarray(
            arr.reshape(NG, GE, P, inner).transpose(2, 0, 1, 3)
            .reshape(P, NG, GE * inner))

    in_maps = []
    for c in range(8):
        qsl = slice(c * 512, (c + 1) * 512)
        ksl = slice(c * 128, (c + 1) * 128)
        wo_c = np.ascontiguousarray(Wo[:, qsl].T).astype(np.float16)  # [512, E]
        wow = np.ascontiguousarray(
            wo_c.reshape(HPC, P, E).transpose(1, 0, 2).reshape(P, HPC * E))
        wkvw = np.ascontiguousarray(np.concatenate(
            [wlayout(Wk[ksl], D).reshape(P, NG, GE, D),
             wlayout(Wv[ksl], D).reshape(P, NG, GE, D)],
            axis=3).reshape(P, NG, GE * 2 * D))
        in_maps.append({
            "hidw": hidw,
            "wqw": wlayout(Wq[qsl], HPC * D),
            "wkvw": wkvw,
            "wow": wow,
            "cosT": cosT,
            "sinT": sinT,
            "mlow": mlow,
            "mup": mup,
        })
    return in_maps


def run(in_maps, **kwargs):
    nc = get_nc()
    return run_bass_kernel_spmd(nc, in_maps, core_ids=list(range(8)), **kwargs)


def kernel(hidden_states, Wq, Wk, Wv, Wo):
    in_maps = make_in_maps(hidden_states, Wq, Wk, Wv, Wo)
    res = run(in_maps)
    out = np.zeros((S, E), dtype=np.float32)
    for r in res.results:
        out += r["out"].astype(np.float32)
    return out.reshape(1, S, E)
